# revision 10
# baseline (speedup 1.0000x reference)
"""Bahdanau attention (with coverage) Trainium2 Bass kernel.

Problem (per full input):
    B, S, D, U = 32, 2048, 1024, 1024
    h     = dec_hidden @ Wh + Wh_b                  [B, U]
    feat  = enc @ Ws + Ws_b + h[:,None,:] + pc @ Wc + Wc_b
    score = tanh(feat) @ V + V_b                    [B, S, 1]
    attn  = softmax(score, axis=1)
    cov   = attn + pc
    ctx   = sum_s attn * enc                        [B, D]

Sharding: data-parallel over batch across 8 NeuronCores (4 rows/core).

Per-core kernel strategy (natural [s-partition, u-free] orientation):
  - enc rows stream in as [128, 1024] tiles; PE-transpose (float32r,
    1.5 cyc/row) into encT [128 d, 512 s] tiles for the feat matmul.
  - feat matmul in float32r (1 cyc/row at N=512 - bf16 speed, fp32 bits).
  - coverage rank-1 term pc[s]*Wc[u] and the per-u bias row are folded
    into the PSUM accumulation as one K=2 matmul: [pc; 1]^T @ [Wc; bias].
  - tanh on ScalarE directly from PSUM -> SBUF.
  - score[s] = sum_u tanh*V via one fused DVE tensor_tensor_reduce.
  - softmax without max subtraction (score = tanh@V is bounded, |score|<~16).
  - context via M=1 float32r matmuls accumulating over s-tiles; enc is
    re-streamed from HBM for this pass (cheaper than keeping it in SBUF).

float32r note: the BIR verifier requires every producer of an f32r matmul
operand to emit f32r, so all matmul-feeding DRAM tensors and SBUF tiles are
declared float32r (same 32-bit data; numpy binds them as float32).
"""

import numpy as np

B, S, D, U = 32, 2048, 1024, 1024
NCORES = 8
BPC = B // NCORES  # batch rows per core
P = 128            # partitions
ST = S // P        # 16 s-tiles per batch row
SCH = 4            # s-chunks per batch row (of 512 rows each)
TPC = ST // SCH    # 4 s-tiles per chunk
KD = D // P        # 8 d-chunks
NH = 2             # u halves of 512

_CACHE = {}


def _build_program():
    import concourse.tile as tile
    from concourse import bacc, mybir

    f32 = mybir.dt.float32
    f32r = mybir.dt.float32r
    AF = mybir.ActivationFunctionType
    ALU = mybir.AluOpType

    nc = bacc.Bacc("TRN2", target_bir_lowering=False, debug=False)

    # ---- I/O (f32r tensors carry plain fp32 bits; numpy binds as float32) ----
    enc = nc.dram_tensor("enc", [BPC, S, D], f32r, kind="ExternalInput").ap()
    dec = nc.dram_tensor("dec", [BPC, D], f32r, kind="ExternalInput").ap()
    pc = nc.dram_tensor("pc", [BPC, S], f32r, kind="ExternalInput").ap()
    pc32 = nc.dram_tensor("pc32", [BPC, S], f32, kind="ExternalInput").ap()
    ws = nc.dram_tensor("ws", [D, U], f32r, kind="ExternalInput").ap()
    wh = nc.dram_tensor("wh", [D, U], f32r, kind="ExternalInput").ap()
    wc = nc.dram_tensor("wc", [U], f32r, kind="ExternalInput").ap()
    bias_comb = nc.dram_tensor("bias_comb", [U], f32r, kind="ExternalInput").ap()
    vrow = nc.dram_tensor("vrow", [U], f32, kind="ExternalInput").ap()
    ident_d = nc.dram_tensor("ident", [P, P], f32r, kind="ExternalInput").ap()
    ident32_d = nc.dram_tensor("ident32", [P, P], f32, kind="ExternalInput").ap()
    ones_col_d = nc.dram_tensor("ones_col", [P, 1], f32r, kind="ExternalInput").ap()
    ones14_d = nc.dram_tensor("ones14", [1, BPC], f32r, kind="ExternalInput").ap()
    ones_s_d = nc.dram_tensor("ones_s", [S], f32r, kind="ExternalInput").ap()

    ctx_out = nc.dram_tensor("ctx", [BPC, U], f32, kind="ExternalOutput").ap()
    attn_out = nc.dram_tensor("attn", [BPC, S], f32, kind="ExternalOutput").ap()
    cov_out = nc.dram_tensor("cov", [BPC, S], f32, kind="ExternalOutput").ap()

    from contextlib import ExitStack
    with tile.TileContext(nc) as tc:
        with ExitStack() as _st:
            singles = _st.enter_context(tc.tile_pool(name="singles", bufs=1))
            whp = _st.enter_context(tc.tile_pool(name="whp", bufs=2))
            encnat = _st.enter_context(tc.tile_pool(name="encnat", bufs=6))
            enctp = _st.enter_context(tc.tile_pool(name="enctp", bufs=12))
            enc2p = _st.enter_context(tc.tile_pool(name="enc2", bufs=6))
            pc2p = _st.enter_context(tc.tile_pool(name="pc2p", bufs=3))
            tanhp = _st.enter_context(tc.tile_pool(name="tanhp", bufs=3))
            ttrs = _st.enter_context(tc.tile_pool(name="ttrs", bufs=2))
            scorep = _st.enter_context(tc.tile_pool(name="scorep", bufs=2))
            smallp = _st.enter_context(tc.tile_pool(name="smallp", bufs=4))
            outp = _st.enter_context(tc.tile_pool(name="outp", bufs=4))
            dramp = _st.enter_context(tc.tile_pool(name="dramp", bufs=1, space="DRAM"))
            psumF = _st.enter_context(tc.tile_pool(name="psumF", bufs=3, space="PSUM"))
            psumT = _st.enter_context(tc.tile_pool(name="psumT", bufs=2, space="PSUM"))
            psumC = _st.enter_context(tc.tile_pool(name="psumC", bufs=2, space="PSUM"))
            psumS = _st.enter_context(tc.tile_pool(name="psumS", bufs=1, space="PSUM"))

            # ---- resident constants ----
            ws_sb = []
            for k in range(KD):
                t = singles.tile([P, U], f32r, tag=f"ws{k}", name=f"ws{k}")
                nc.sync.dma_start(out=t[:], in_=ws[k * P:(k + 1) * P, :])
                ws_sb.append(t)
            vb_sb = singles.tile([P, U], f32, tag="vb")
            nc.gpsimd.dma_start(out=vb_sb[:], in_=vrow.unsqueeze(0).to_broadcast([P, U]))
            ident = singles.tile([P, P], f32r, tag="ident")
            nc.sync.dma_start(out=ident[:], in_=ident_d[:])
            ident32 = singles.tile([P, P], f32, tag="ident32")
            nc.sync.dma_start(out=ident32[:], in_=ident32_d[:])
            ones_col = singles.tile([P, 1], f32r, tag="ones_col")
            nc.sync.dma_start(out=ones_col[:], in_=ones_col_d[:])
            ones14 = singles.tile([1, BPC], f32r, tag="ones14")
            nc.sync.dma_start(out=ones14[:], in_=ones14_d[:])
            biasc_sb = singles.tile([1, U], f32r, tag="biasc")
            nc.sync.dma_start(out=biasc_sb[:], in_=bias_comb.unsqueeze(0))
            dect = singles.tile([P, KD * BPC], f32r, tag="dect")
            for k in range(KD):
                nc.sync.dma_start(
                    out=dect[:, k * BPC:(k + 1) * BPC],
                    in_=dec[:, k * P:(k + 1) * P].rearrange("b d -> d b"),
                )

            # ---- h = dec @ Wh + (Ws_b + Wc_b + Wh_b): [BPC, U] ----
            psh = [psumC.tile([BPC, 512], f32, tag="ctx", name=f"psh{_h}")
                   for _h in range(NH)]
            for k in range(KD):
                wh_t = whp.tile([P, U], f32r, tag="wht")
                nc.sync.dma_start(out=wh_t[:], in_=wh[k * P:(k + 1) * P, :])
                for h in range(NH):
                    nc.tensor.matmul(
                        psh[h][:],
                        lhsT=dect[:, k * BPC:(k + 1) * BPC],
                        rhs=wh_t[:, h * 512:(h + 1) * 512],
                        start=(k == 0), stop=False,
                    )
            for h in range(NH):
                nc.tensor.matmul(
                    psh[h][:],
                    lhsT=ones14[:],
                    rhs=biasc_sb[:, h * 512:(h + 1) * 512],
                    start=False, stop=True,
                )
            hsb = smallp.tile([BPC, U], f32r, tag="hsb", bufs=1)
            for h in range(NH):
                nc.scalar.copy(hsb[:, h * 512:(h + 1) * 512], psh[h][:])
            h_dram = dramp.tile([BPC, U], f32r, tag="hdram")
            nc.sync.dma_start(out=h_dram[:], in_=hsb[:])

            # ---- per-batch [Wc; bias] K=2 rhs, and pc^T for coverage ----
            rank2_sb, pcT_sb = [], []
            for b in range(BPC):
                t2 = singles.tile([2, U], f32r, tag=f"rank2_{b}", name=f"rank2_{b}")
                nc.sync.dma_start(out=t2[0:1, :], in_=wc.unsqueeze(0))
                nc.sync.dma_start(out=t2[1:2, :], in_=h_dram[b].unsqueeze(0))
                rank2_sb.append(t2)
                t3 = singles.tile([P, ST], f32, tag=f"pcT_{b}", name=f"pcT_{b}")
                nc.sync.dma_start(
                    out=t3[:], in_=pc32[b].rearrange("(t p) -> p t", p=P))
                pcT_sb.append(t3)

            # ---- main per-batch-row pipeline ----
            for b in range(BPC):
                score_sb = scorep.tile([P, ST], f32, tag="score")
                for n in range(SCH):
                    # [pc; 1] K=2 lhsT rows for this 512-row s-chunk
                    pc2t = pc2p.tile([2, 512], f32r, tag="pc2t", name="pc2t")
                    nc.sync.dma_start(
                        out=pc2t[0:1, :],
                        in_=pc[b, n * 512:(n + 1) * 512].unsqueeze(0))
                    nc.sync.dma_start(
                        out=pc2t[1:2, :],
                        in_=ones_s_d[n * 512:(n + 1) * 512].unsqueeze(0))
                    # load 4 natural enc tiles for this 512-row s-chunk
                    etiles = []
                    for i in range(TPC):
                        s0 = (n * TPC + i) * P
                        et = encnat.tile([P, D], f32r, tag="encnat",
                                         name=f"encnat{i}")
                        nc.sync.dma_start(out=et[:], in_=enc[b, s0:s0 + P, :])
                        etiles.append(et)
                    # transpose to encT[k] = [128 d, 512 s]
                    enct = []
                    for k in range(KD):
                        pst = psumT.tile([P, 512], f32, tag="pst")
                        for i in range(TPC):
                            nc.tensor.transpose(
                                pst[:, i * P:(i + 1) * P].bitcast(f32r),
                                etiles[i][:, k * P:(k + 1) * P],
                                ident[:],
                            )
                        ek = enctp.tile([P, 512], f32r, tag="enct",
                                        name=f"enct{k}")
                        nc.vector.tensor_copy(ek[:], pst[:])
                        enct.append(ek)
                    # feat matmul + tanh + score for the 4 s-tiles of the chunk
                    for i in range(TPC):
                        t_idx = n * TPC + i
                        psf = [psumF.tile([P, 512], f32, tag="psf",
                                          name=f"psf{_h}") for _h in range(NH)]
                        for k in range(KD):
                            lhs = enct[k][:, i * P:(i + 1) * P]
                            for h in range(NH):
                                nc.tensor.matmul(
                                    psf[h][:],
                                    lhsT=lhs,
                                    rhs=ws_sb[k][:, h * 512:(h + 1) * 512],
                                    start=(k == 0), stop=False,
                                )
                        for h in range(NH):
                            nc.tensor.matmul(
                                psf[h][:],
                                lhsT=pc2t[:, i * P:(i + 1) * P],
                                rhs=rank2_sb[b][:, h * 512:(h + 1) * 512],
                                start=False, stop=True,
                            )
                        tnh = tanhp.tile([P, U], f32, tag="tnh")
                        for h in range(NH):
                            nc.scalar.activation(
                                tnh[:, h * 512:(h + 1) * 512], psf[h][:], AF.Tanh,
                            )
                        prod = ttrs.tile([P, U], f32, tag="tout")
                        nc.vector.tensor_mul(prod[:], tnh[:], vb_sb[:])
                        trash = ttrs.tile([P, U], f32, tag="trash")
                        nc.scalar.activation(
                            trash[:], prod[:], AF.Copy,
                            accum_out=score_sb[:, t_idx:t_idx + 1],
                        )

                # ---- softmax (no max-subtraction; scores are bounded) ----
                e_sb = scorep.tile([P, ST], f32r, tag="esb")
                nc.scalar.activation(e_sb[:], score_sb[:], AF.Exp)
                e32 = scorep.tile([P, ST], f32, tag="e32")
                nc.scalar.activation(e32[:], score_sb[:], AF.Exp)
                # Z = sum(e): column sums via ones^T @ e, then free-dim reduce
                psz = psumS.tile([1, ST], f32, tag="small")
                nc.tensor.matmul(psz[:], lhsT=ones_col[:], rhs=e_sb[:],
                                 start=True, stop=True)
                zsum = smallp.tile([1, 1], f32, tag="zsum")
                nc.vector.reduce_sum(out=zsum[:], in_=psz[:],
                                     axis=mybir.AxisListType.X)
                rec = smallp.tile([1, 1], f32, tag="rec")
                nc.vector.reciprocal(rec[:], zsum[:])
                rec_dram = dramp.tile([1, 1], f32, tag="rec_dram", name="rec_dram")
                nc.sync.dma_start(out=rec_dram[:], in_=rec[:])
                recb = smallp.tile([P, 1], f32, tag="recb")
                nc.gpsimd.dma_start(out=recb[:],
                                    in_=rec_dram[0:1, 0:1].to_broadcast([P, 1]))

                attn_sb = smallp.tile([P, ST], f32, tag="attn_sb")
                nc.vector.tensor_scalar_mul(attn_sb[:], e32[:], recb[:])
                cov_sb = smallp.tile([P, ST], f32, tag="cov_sb")
                nc.vector.tensor_add(cov_sb[:], attn_sb[:], pcT_sb[b][:])

                # transpose [128, 16] -> [16, 128] so DMA out is contiguous
                psa = psumS.tile([ST, P], f32, tag="small")
                nc.tensor.transpose(psa[:], attn_sb[:], ident32[:])
                attnT = outp.tile([ST, P], f32, tag="attnT")
                nc.scalar.copy(attnT[:], psa[:])
                nc.sync.dma_start(
                    out=attn_out[b].rearrange("(t p) -> t p", p=P), in_=attnT[:])
                psc2 = psumS.tile([ST, P], f32, tag="small")
                nc.tensor.transpose(psc2[:], cov_sb[:], ident32[:])
                covT = outp.tile([ST, P], f32, tag="covT")
                nc.scalar.copy(covT[:], psc2[:])
                nc.sync.dma_start(
                    out=cov_out[b].rearrange("(t p) -> t p", p=P), in_=covT[:])

                # ---- context: ctx[d] = (1/Z) * sum_s e[s] * enc[s, d] ----
                psc = [psumC.tile([1, 512], f32, tag="ctx", name=f"psc{_h}")
                       for _h in range(NH)]
                for t in range(ST):
                    e2 = enc2p.tile([P, D], f32r, tag="enc2", name="enc2")
                    nc.sync.dma_start(out=e2[:], in_=enc[b, t * P:(t + 1) * P, :])
                    for h in range(NH):
                        nc.tensor.matmul(
                            psc[h][:],
                            lhsT=e_sb[:, t:t + 1],
                            rhs=e2[:, h * 512:(h + 1) * 512],
                            start=(t == 0), stop=(t == ST - 1),
                        )
                ctx_sb = outp.tile([1, U], f32, tag="ctx_sb", bufs=2)
                for h in range(NH):
                    nc.scalar.activation(
                        ctx_sb[:, h * 512:(h + 1) * 512], psc[h][:], AF.Copy,
                        scale=rec[:],
                    )
                nc.sync.dma_start(out=ctx_out[b].unsqueeze(0), in_=ctx_sb[:])

    nc.compile()
    return nc


def _get_program():
    if "nc" not in _CACHE:
        _CACHE["nc"] = _build_program()
    return _CACHE["nc"]


def make_in_maps(enc_output, dec_hidden, pc_full, Ws_w, Wh_w, wc_row, bias_comb,
                 v_row):
    in_maps = []
    for c in range(NCORES):
        sl = slice(c * BPC, (c + 1) * BPC)
        in_maps.append({
            "enc": np.ascontiguousarray(enc_output[sl]),
            "dec": np.ascontiguousarray(dec_hidden[sl]),
            "pc": np.ascontiguousarray(pc_full[sl]),
            "pc32": np.ascontiguousarray(pc_full[sl]),
            "ws": Ws_w,
            "wh": Wh_w,
            "wc": wc_row,
            "bias_comb": bias_comb,
            "vrow": v_row,
            "ident": np.eye(P, dtype=np.float32),
            "ident32": np.eye(P, dtype=np.float32),
            "ones_col": np.ones((P, 1), np.float32),
            "ones14": np.ones((1, BPC), np.float32),
            "ones_s": np.ones((S,), np.float32),
        })
    return in_maps


def _prep(dec_hidden, enc_output, use_coverage, prev_coverage,
          Ws_w, Ws_b, Wh_w, Wh_b, Wc_w, Wc_b, V_w):
    dec_hidden = np.asarray(dec_hidden, dtype=np.float32)
    enc_output = np.asarray(enc_output, dtype=np.float32)
    Ws_w = np.ascontiguousarray(np.asarray(Ws_w, dtype=np.float32))
    Wh_w = np.ascontiguousarray(np.asarray(Wh_w, dtype=np.float32))

    use_cov = bool(use_coverage)
    has_pc = use_cov and prev_coverage is not None
    if has_pc:
        pc_full = np.ascontiguousarray(
            np.asarray(prev_coverage, dtype=np.float32).reshape(B, S))
    else:
        pc_full = np.zeros((B, S), dtype=np.float32)

    # Host-side weight prep (O(U)). Wh_b folds into the bias row; V_b only
    # shifts `score` uniformly, which cancels in the softmax.
    bias_comb = (np.asarray(Ws_b, np.float32) + np.asarray(Wc_b, np.float32)
                 + np.asarray(Wh_b, np.float32)).astype(np.float32)
    wc_row = np.asarray(Wc_w, np.float32).reshape(U)
    v_row = np.asarray(V_w, np.float32).reshape(U)
    return (dec_hidden, enc_output, pc_full, Ws_w, Wh_w, wc_row, bias_comb,
            v_row, use_cov, has_pc)


def kernel(dec_hidden, enc_output, enc_pad_mask=None, use_coverage=True,
           prev_coverage=None, Ws_w=None, Ws_b=None, Wh_w=None, Wh_b=None,
           Wc_w=None, Wc_b=None, V_w=None, V_b=None, **_unused):
    from concourse.bass_utils import run_bass_kernel_spmd

    (dec_hidden, enc_output, pc_full, Ws_w, Wh_w, wc_row, bias_comb, v_row,
     use_cov, has_pc) = _prep(dec_hidden, enc_output, use_coverage,
                              prev_coverage, Ws_w, Ws_b, Wh_w, Wh_b, Wc_w,
                              Wc_b, V_w)

    nc = _get_program()
    in_maps = make_in_maps(enc_output, dec_hidden, pc_full, Ws_w, Wh_w, wc_row,
                           bias_comb, v_row)
    res = run_bass_kernel_spmd(nc, in_maps, core_ids=list(range(NCORES))).results

    ctx = np.concatenate([res[c]["ctx"] for c in range(NCORES)], axis=0)
    attn = np.concatenate([res[c]["attn"] for c in range(NCORES)], axis=0)
    cov = np.concatenate([res[c]["cov"] for c in range(NCORES)], axis=0)

    if not use_cov:
        coverage = []
    elif has_pc:
        coverage = cov.reshape(B, S, 1)
    else:
        coverage = attn.reshape(B, S, 1)
    return ctx, attn, coverage
